# revision 1
# baseline (speedup 1.0000x reference)
"""Trainium2 Bass kernel for nn_FDAF (concat -> depthwise5x5 -> InstanceNorm ->
GELU -> 1x1 conv -> bilinear warp -> subtract), data-parallel over 8 cores.

Sharding: core c = (sample n = c//2, row-half s = c%2). Each core computes both
feature outputs for its 128-row half of its sample. InstanceNorm stats are
combined across the row-half pair with a tiny AllReduce collective.

Self-contained: hardcodes N=4, C=64, H=W=256.
"""
import numpy as np
import ml_dtypes

import concourse.bass as bass
import concourse.bacc as bacc
import concourse.tile as tile
from concourse import mybir
from concourse.bass_utils import run_bass_kernel_spmd

FP32 = mybir.dt.float32
BF16 = mybir.dt.bfloat16
AL = mybir.AluOpType
AF = mybir.ActivationFunctionType
AX = mybir.AxisListType

N, C, H, W = 4, 64, 256, 256
HH = 128          # rows per core (half image)
RT = 8            # rows per tile
NT = HH // RT     # 16 tiles
FT = RT * W       # free elems per tile = 2048
FH = HH * W       # free elems per half = 32768
DSCALE = (W - 1) / (2.0 * W)  # flow -> pixel displacement (align_corners=True)

# tap split for the depthwise conv (25 taps, k = dy*5+dx).
# DVE gets even-dx taps only: odd-dx reads are 2-byte-misaligned in the bf16
# slab and would fall out of the DVE 4x/2x packing modes on real hardware;
# PE/ACT/GPSIMD reads are alignment-insensitive.
_EVEN = [k for k in range(25) if (k % 5) % 2 == 0]   # 15 taps
_ODD = [k for k in range(25) if (k % 5) % 2 == 1]    # 10 taps
DVE_TAPS = _EVEN + _ODD[:1]        # 16 taps (8 pairs; one odd tap tolerated)
PE_TAPS = _ODD[1:4]                # 3 diag-matmul direct taps
ACT_TAPS = _ODD[4:8]               # 4 scale-copy products (2 pairs)
GPS_TAPS = _ODD[8:10]              # 2 ts products (1 pair, DVE-added)

_CACHE = {}


def _build(timing=False):
    nc = bacc.Bacc("TRN2", target_bir_lowering=False, debug=False,
                   num_devices=1 if timing else 8)

    xh = nc.dram_tensor("xh", [128, 132, 260], BF16, kind="ExternalInput")
    wb = nc.dram_tensor("wb", [128, 26], FP32, kind="ExternalInput")
    pw = nc.dram_tensor("pw", [128, 4], BF16, kind="ExternalInput")
    dg = nc.dram_tensor("dg", [len(PE_TAPS) * 128, 128], BF16, kind="ExternalInput")
    ident = nc.dram_tensor("ident", [128, 128], BF16, kind="ExternalInput")
    nsw = nc.dram_tensor("nsw", [128, 128], BF16, kind="ExternalInput")
    out_d = nc.dram_tensor("out", [128, FH], FP32, kind="ExternalOutput")

    cc_in = nc.dram_tensor("cc_in", [128, 2], FP32, kind="Internal")
    cc_out = nc.dram_tensor("cc_out", [128, 2], FP32, kind="Internal")
    FH2 = FH // 2
    flow_hs = [nc.dram_tensor(f"flow_d{h}", [4, FH2], BF16, kind="Internal")
               for h in range(2)]
    WL = 64 * 260 + 8   # padded-row weight map: 260-stride rows + slack
    w9_hs = [nc.dram_tensor(f"w9_d{h}", [18, WL], BF16, kind="Internal")
             for h in range(2)]

    with tile.TileContext(nc) as tc:
        with tc.tile_pool(name="singles", bufs=1) as singles:
            xh_sb = singles.tile([128, 132, 260], BF16)
            for ci in range(11):
                r0c, r1c = ci * 12, min(132, ci * 12 + 12)
                eng = (nc.sync, nc.scalar)[ci % 2]
                eng.dma_start(out=xh_sb[:, r0c:r1c, :],
                              in_=xh.ap()[:, r0c:r1c, :])
            wb_sb = singles.tile([128, 26], FP32)
            nc.sync.dma_start(out=wb_sb, in_=wb.ap())
            pw_sb = singles.tile([128, 4], BF16)
            nc.sync.dma_start(out=pw_sb, in_=pw.ap())
            id_sb = singles.tile([128, 128], BF16)
            nc.sync.dma_start(out=id_sb, in_=ident.ap())
            nsw_sb = singles.tile([128, 128], BF16)
            nc.sync.dma_start(out=nsw_sb, in_=nsw.ap())
            dg_sb = singles.tile([128, len(PE_TAPS), 128], BF16)
            nc.sync.dma_start(out=dg_sb,
                              in_=dg.ap().rearrange("(k p) m -> p k m", p=128))
            eps_t = singles.tile([128, 1], FP32)
            nc.vector.memset(eps_t, 1e-5)
            sp = singles.tile([128, NT], FP32)
            s2p = singles.tile([128, NT], FP32)
            stat = singles.tile([128, 8], FP32)

            # ---------------- Phase A: conv + stats; Y stored bf16 -------------
            with tc.tile_pool(name="ab", bufs=1) as ab:
              y_bf = ab.tile([128, FH], BF16)
              with tc.tile_pool(name="pha", bufs=2) as pha, \
                   tc.tile_pool(name="psA", bufs=2, space="PSUM") as psA:
                for t in range(NT):
                    r0 = t * RT

                    def sl(dy, dx, r0=r0):
                        return xh_sb[:, r0 + dy:r0 + RT + dy, dx:dx + W]

                    # DVE tap pairs: ts product x2 (4x mode) + TT pair-add,
                    # each pair merged into psum by one PE identity pass
                    ps = psA.tile([128, FT], FP32)
                    nch = FT // 512
                    for ki, k in enumerate(PE_TAPS):
                        dy, dx = divmod(k, 5)
                        for j in range(nch):
                            nc.tensor.matmul(
                                ps[:, j * 512:(j + 1) * 512], dg_sb[:, ki, :],
                                xh_sb[:, r0 + dy + 2 * j:r0 + dy + 2 * j + 2,
                                      dx:dx + W],
                                start=(ki == 0), stop=False)

                    def pe_merge(src3d, last=False):
                        for j in range(nch):
                            nc.tensor.matmul(
                                ps[:, j * 512:(j + 1) * 512], id_sb,
                                src3d[:, 2 * j:2 * j + 2, :],
                                start=False, stop=(last and j == nch - 1))

                    npair = len(DVE_TAPS) // 2
                    pairs = []
                    for pi in range(npair):
                        ka, kb = DVE_TAPS[2 * pi], DVE_TAPS[2 * pi + 1]
                        pa = pha.tile([128, RT, W], BF16, tag=f"pa{pi % 2}",
                                      bufs=3)
                        pb = pha.tile([128, RT, W], BF16, tag="pb", bufs=3)
                        # fold the conv bias into the first product
                        if pi == 0:
                            nc.vector.tensor_scalar(
                                out=pa, in0=sl(*divmod(ka, 5)),
                                scalar1=wb_sb[:, ka:ka + 1],
                                scalar2=wb_sb[:, 25:26],
                                op0=AL.mult, op1=AL.add)
                        else:
                            nc.vector.tensor_scalar(
                                out=pa, in0=sl(*divmod(ka, 5)),
                                scalar1=wb_sb[:, ka:ka + 1], scalar2=None,
                                op0=AL.mult)
                        nc.vector.tensor_scalar(
                            out=pb, in0=sl(*divmod(kb, 5)),
                            scalar1=wb_sb[:, kb:kb + 1], scalar2=None, op0=AL.mult)
                        if pi < npair - 4:
                            nc.vector.tensor_tensor(out=pa, in0=pa, in1=pb,
                                                    op=AL.add)
                            pairs.append(pa)
                        else:
                            pairs.append(pa)
                            pairs.append(pb)
                    for m in pairs:
                        pe_merge(m)

                    # ACT products (scale-copy), pair-added on GPSIMD
                    aps = []
                    for ai, k in enumerate(ACT_TAPS):
                        at = pha.tile([128, RT, W], BF16, tag=f"at{ai % 2}")
                        nc.scalar.mul(out=at, in_=sl(*divmod(k, 5)),
                                      mul=wb_sb[:, k:k + 1])
                        aps.append(at)
                    nc.gpsimd.tensor_tensor(out=aps[0], in0=aps[0], in1=aps[1],
                                            op=AL.add)
                    nc.gpsimd.tensor_tensor(out=aps[2], in0=aps[2], in1=aps[3],
                                            op=AL.add)
                    pe_merge(aps[0])
                    pe_merge(aps[2])

                    # GPSIMD ts pair, DVE-added
                    ka, kb = GPS_TAPS
                    ga = pha.tile([128, RT, W], BF16, tag="ga")
                    gb = pha.tile([128, RT, W], BF16, tag="gb")
                    nc.gpsimd.tensor_scalar(out=ga, in0=sl(*divmod(ka, 5)),
                                            scalar1=wb_sb[:, ka:ka + 1],
                                            scalar2=None, op0=AL.mult)
                    nc.gpsimd.tensor_scalar(out=gb, in0=sl(*divmod(kb, 5)),
                                            scalar1=wb_sb[:, kb:kb + 1],
                                            scalar2=None, op0=AL.mult)
                    nc.vector.tensor_tensor(out=ga, in0=ga, in1=gb, op=AL.add)
                    pe_merge(ga, last=True)

                    # stats from psum + store bf16 (Copy pass writes Y;
                    # Square scratch writes the same region first, WAW-ordered)
                    nc.scalar.activation(out=y_bf[:, t * FT:(t + 1) * FT], in_=ps,
                                         func=AF.Square,
                                         accum_out=s2p[:, t:t + 1])
                    nc.scalar.activation(out=y_bf[:, t * FT:(t + 1) * FT], in_=ps,
                                         func=AF.Copy, accum_out=sp[:, t:t + 1])

              # ------------- stats combine (collective over the pair) -----------
              nc.vector.tensor_reduce(out=stat[:, 0:1], in_=sp, axis=AX.X, op=AL.add)
              nc.vector.tensor_reduce(out=stat[:, 1:2], in_=s2p, axis=AX.X, op=AL.add)
              nc.sync.dma_start(out=cc_in.ap(), in_=stat[:, 0:2])
              if not timing:
                  nc.gpsimd.collective_compute(
                      kind="AllReduce", op=AL.add,
                      replica_groups=[[0, 1], [2, 3], [4, 5], [6, 7]],
                      ins=[cc_in.ap()], outs=[cc_out.ap()])
              else:
                  nc.sync.dma_start(out=cc_out.ap(), in_=cc_in.ap())
              nc.sync.dma_start(out=stat[:, 0:2], in_=cc_out.ap())
              inv = 1.0 / (H * W)
              nc.vector.tensor_scalar(out=stat[:, 2:3], in0=stat[:, 0:1],
                                      scalar1=inv, scalar2=None, op0=AL.mult)
              nc.vector.tensor_scalar(out=stat[:, 3:4], in0=stat[:, 1:2],
                                      scalar1=inv, scalar2=None, op0=AL.mult)
              nc.vector.scalar_tensor_tensor(
                  out=stat[:, 4:5], in0=stat[:, 2:3], scalar=stat[:, 2:3],
                  in1=stat[:, 3:4], op0=AL.mult, op1=AL.subtract)
              nc.scalar.activation(out=stat[:, 5:6], in_=stat[:, 4:5],
                                   func=AF.Sqrt, scale=-1.0, bias=eps_t[:, 0:1])
              nc.vector.reciprocal(out=stat[:, 6:7], in_=stat[:, 5:6])
              nc.vector.tensor_scalar(out=stat[:, 7:8], in0=stat[:, 2:3],
                                      scalar1=stat[:, 6:7], scalar2=-1.0,
                                      op0=AL.mult, op1=AL.mult)

              # --------------- Phase B: gelu + 1x1 conv -> flow_d ---------------
              with tc.tile_pool(name="phb", bufs=2) as phb, \
                   tc.tile_pool(name="psB", bufs=2, space="PSUM") as psB:
                  for t in range(NT):
                      h, tl = t // (NT // 2), t % (NT // 2)
                      g = phb.tile([128, FT], BF16)
                      nc.scalar.activation(out=g, in_=y_bf[:, t * FT:(t + 1) * FT],
                                           func=AF.Gelu, scale=stat[:, 6:7],
                                           bias=stat[:, 7:8])
                      psf = psB.tile([4, FT], FP32)
                      for j in range(FT // 512):
                          nc.tensor.matmul(psf[:, j * 512:(j + 1) * 512], pw_sb,
                                           g[:, j * 512:(j + 1) * 512],
                                           start=True, stop=True)
                      fls = phb.tile([4, FT], BF16)
                      nc.scalar.copy(out=fls, in_=psf)
                      nc.sync.dma_start(
                          out=flow_hs[h].ap()[:, tl * FT:(tl + 1) * FT], in_=fls)

            # -------- compact weight maps: deltas -> 18 tap products ----------
            # per half: cx/cy [64, FH2/32]; part 0-31 field1, 32-63 field2
            with tc.tile_pool(name="cw", bufs=2) as cw:
              for h in range(2):
                FC = FH2 // 32
                cx = cw.tile([64, FC], BF16, tag="cx")
                cy = cw.tile([64, FC], BF16, tag="cy")
                for (dst, r1, r2) in ((cx, 0, 2), (cy, 1, 3)):
                    for (p0, row) in ((0, r1), (32, r2)):
                        nc.sync.dma_start(
                            out=dst[p0:p0 + 32, :],
                            in_=flow_hs[h].ap()[row:row + 1, :].rearrange(
                                "a (p f) -> (a p) f", p=32))
                wsel = {}
                for ax, d in (("x", cx), ("y", cy)):
                    wp = cw.tile([64, FC], BF16, tag=f"wp{ax}")
                    wm = cw.tile([64, FC], BF16, tag=f"wm{ax}")
                    w0 = cw.tile([64, FC], BF16, tag=f"w0{ax}")
                    nc.vector.tensor_scalar(out=wp, in0=d, scalar1=0.0,
                                            scalar2=None, op0=AL.max)
                    nc.vector.tensor_scalar(out=wm, in0=d, scalar1=-1.0,
                                            scalar2=0.0, op0=AL.mult, op1=AL.max)
                    nc.scalar.activation(out=w0, in_=d, func=AF.Abs)
                    nc.vector.tensor_scalar(out=w0, in0=w0, scalar1=-1.0,
                                            scalar2=1.0, op0=AL.mult, op1=AL.add)
                    wsel[ax] = {-1: wm, 0: w0, 1: wp}
                for ki, (sy, sx) in enumerate(
                        (sy, sx) for sy in (-1, 0, 1) for sx in (-1, 0, 1)):
                    p9 = cw.tile([64, FC], BF16, tag="p9")
                    nc.vector.tensor_tensor(out=p9, in0=wsel["y"][sy],
                                            in1=wsel["x"][sx], op=AL.mult)
                    wd = w9_hs[h].ap()
                    for f in range(2):
                        dst = bass.AP(tensor=wd.tensor,
                                      offset=(ki + 9 * f) * WL + 6,
                                      ap=[[520, 32], [260, 2], [1, 256]])
                        nc.sync.dma_start(out=dst, in_=p9[32 * f:32 * f + 32, :])

            # ---------------- Phase C: warp + subtract ----------------
            with tc.tile_pool(name="phc", bufs=2) as phc, \
                 tc.tile_pool(name="psC", bufs=2, space="PSUM") as psC:
                taps = [(sy, sx) for sy in (-1, 0, 1) for sx in (-1, 0, 1)]
                for t in range(NT):
                    h, tl = t // (NT // 2), t % (NT // 2)
                    w9 = w9_hs[h].ap()
                    r0 = t * RT
                    acc = psC.tile([128, FT], FP32)
                    nch = FT // 512

                    RW = RT * 260

                    def wtile(ki, sx, w9=w9, tl=tl):
                        # pre-shifted by -sx: wt[., r, c] = w9[y=tl*8+r, c-2-sx]
                        wt = phc.tile([128, RT, 260], BF16, tag=f"w9t{ki % 4}",
                                      bufs=3)
                        src = bass.AP(tensor=w9.tensor,
                                      offset=ki * WL + 4 - sx + tl * RW,
                                      ap=[[9 * WL, 2], [0, 64], [1, RW]])
                        eng = {0: nc.sync, 3: nc.sync, 1: nc.scalar,
                               4: nc.scalar}.get(ki, nc.gpsimd)
                        eng.dma_start(out=wt, in_=src)
                        return wt

                    # 9 mults on DVE over full aligned 260-wide rows; the
                    # pixel shift happens in the PE rhs read (alignment-free).
                    # Each 512-col PSUM bank needs its own start=True on tap 0.
                    for ki in range(9):
                        sy, sx = taps[ki]
                        tcl = phc.tile([128, RT, 260], BF16, tag=f"ts{ki % 4}")
                        nc.vector.tensor_tensor(
                            out=tcl, in0=wtile(ki, sx),
                            in1=xh_sb[:, r0 + 2 + sy:r0 + 2 + RT + sy, 0:260],
                            op=AL.mult)
                        for j in range(nch):
                            nc.tensor.matmul(
                                acc[:, j * 512:(j + 1) * 512], id_sb,
                                tcl[:, 2 * j:2 * j + 2, 2 + sx:258 + sx],
                                start=(ki == 0), stop=False)
                    # subtract swapped-half center via permuted negative identity
                    for j in range(nch):
                        nc.tensor.matmul(acc[:, j * 512:(j + 1) * 512], nsw_sb,
                                         xh_sb[:, r0 + 2 + 2 * j:r0 + 4 + 2 * j,
                                               2:2 + W],
                                         start=False, stop=True)
                    outs = phc.tile([128, FT], FP32)
                    nc.scalar.copy(out=outs, in_=acc)
                    nc.gpsimd.dma_start(out=out_d.ap()[:, t * FT:(t + 1) * FT],
                                        in_=outs)
    nc.compile()
    return nc


def _prep_inputs(x1, x2, dw_w, dw_b, pw_w):
    bf = ml_dtypes.bfloat16
    xcat = np.concatenate([x1, x2], axis=1)  # [N,128,H,W] f32
    xpad = np.pad(xcat, ((0, 0), (0, 0), (2, 2), (2, 2))).astype(bf)
    wb = np.concatenate([dw_w.reshape(128, 25), dw_b.reshape(128, 1)],
                        axis=1).astype(np.float32)
    pwm = (pw_w.reshape(4, 128).T * DSCALE).astype(bf)  # [128,4]
    dgm = np.zeros((len(PE_TAPS) * 128, 128), dtype=bf)
    for ki, k in enumerate(PE_TAPS):
        dy, dx = divmod(k, 5)
        np.fill_diagonal(dgm[ki * 128:(ki + 1) * 128], dw_w[:, 0, dy, dx].astype(bf))
    idm = np.eye(128, dtype=bf)
    nswm = np.zeros((128, 128), dtype=bf)
    for m in range(128):
        nswm[(m + 64) % 128, m] = -1.0
    in_maps = []
    for c in range(8):
        n, s = c // 2, c % 2
        in_maps.append({
            "xh": np.ascontiguousarray(xpad[n, :, 128 * s:128 * s + 132, :]),
            "wb": wb, "pw": pwm, "dg": dgm, "ident": idm, "nsw": nswm,
        })
    return in_maps


def _run(x1, x2, dw_w, dw_b, pw_w, trace=False):
    if "nc" not in _CACHE:
        _CACHE["nc"] = _build()
    in_maps = _prep_inputs(np.asarray(x1, np.float32), np.asarray(x2, np.float32),
                           np.asarray(dw_w, np.float32), np.asarray(dw_b, np.float32),
                           np.asarray(pw_w, np.float32))
    res = run_bass_kernel_spmd(_CACHE["nc"], in_maps, core_ids=list(range(8)),
                               trace=trace)
    o1 = np.empty((N, C, H, W), np.float32)
    o2 = np.empty((N, C, H, W), np.float32)
    for c in range(8):
        n, s = c // 2, c % 2
        o = res.results[c]["out"].reshape(128, HH, W)
        o1[n, :, 128 * s:128 * (s + 1), :] = o[:64]
        o2[n, :, 128 * s:128 * (s + 1), :] = o[64:]
    return (o1, o2), res


def kernel(x1, x2, dw_w, dw_b, pw_w):
    (o1, o2), _ = _run(x1, x2, dw_w, dw_b, pw_w, trace=False)
    return (o1, o2)



# revision 33
# speedup vs baseline: 1.0910x; 1.0910x over previous
"""Trainium2 Bass kernel for nn_FDAF (concat -> depthwise5x5 -> InstanceNorm ->
GELU -> 1x1 conv -> bilinear warp -> subtract), data-parallel over 8 cores.

Sharding: core c = (sample n = c//2, row-half s = c%2). Each core computes both
feature outputs for its 128-row half of its sample. InstanceNorm stats are
combined across the row-half pair with a tiny AllReduce collective.

Self-contained: hardcodes N=4, C=64, H=W=256.
"""
import numpy as np
import ml_dtypes

import concourse.bass as bass
import concourse.bacc as bacc
import concourse.tile as tile
from concourse import mybir
from concourse.bass_utils import run_bass_kernel_spmd

FP32 = mybir.dt.float32
BF16 = mybir.dt.bfloat16
AL = mybir.AluOpType
AF = mybir.ActivationFunctionType
AX = mybir.AxisListType

N, C, H, W = 4, 64, 256, 256
HH = 128          # rows per core (half image)
RT = 8            # rows per tile
NT = HH // RT     # 16 tiles
FT = RT * W       # free elems per tile = 2048
FH = HH * W       # free elems per half = 32768
DSCALE = (W - 1) / (2.0 * W)  # flow -> pixel displacement (align_corners=True)

# tap split for the depthwise conv (25 taps, k = dy*5+dx).
# DVE gets even-dx taps only: odd-dx reads are 2-byte-misaligned in the bf16
# slab and would fall out of the DVE 4x/2x packing modes on real hardware;
# PE/ACT/GPSIMD reads are alignment-insensitive.
_EVEN = [k for k in range(25) if (k % 5) % 2 == 0]   # 15 taps
_ODD = [k for k in range(25) if (k % 5) % 2 == 1]    # 10 taps
DVE_TAPS = _EVEN + _ODD[:1]        # 16 taps (8 pairs; one odd tap tolerated)
PE_TAPS = _ODD[1:4]                # 3 diag-matmul direct taps
ACT_TAPS = _ODD[4:8]               # 4 products (1 psum-init + pair + single)
GPS_TAPS = _ODD[8:10]              # 2 ts products (1 pair, DVE-added)
PE_BCAST = {2, 5, 8}               # warp maps broadcast by PE instead of DMA

_CACHE = {}


def _build(timing=False):
    nc = bacc.Bacc("TRN2", target_bir_lowering=False, debug=False,
                   num_devices=1 if timing else 8)

    xh = nc.dram_tensor("xh", [128, 132, 260], BF16, kind="ExternalInput")
    wb = nc.dram_tensor("wb", [128, 26], FP32, kind="ExternalInput")
    pw = nc.dram_tensor("pw", [128, 4], BF16, kind="ExternalInput")
    dg = nc.dram_tensor("dg", [len(PE_TAPS) * 128, 128], BF16, kind="ExternalInput")
    ident = nc.dram_tensor("ident", [128, 128], BF16, kind="ExternalInput")
    nsw = nc.dram_tensor("nsw", [128, 128], BF16, kind="ExternalInput")
    sel = nc.dram_tensor("sel", [64, 32 * 128], BF16, kind="ExternalInput")
    out_d = nc.dram_tensor("out", [128, FH], BF16, kind="ExternalOutput")

    cc_in = nc.dram_tensor("cc_in", [128, 2], FP32, kind="Internal")
    cc_out = nc.dram_tensor("cc_out", [128, 2], FP32, kind="Internal")
    FH2 = FH // 2
    flow_hs = [nc.dram_tensor(f"flow_d{h}", [4, FH2], BF16, kind="Internal")
               for h in range(2)]
    y_d = nc.dram_tensor("y_d", [128, FH // 2], BF16, kind="Internal")
    WL = 64 * 256    # packed weight map: 256-packed rows, pixel = y*256+x
    w9_hs = [nc.dram_tensor(f"w9_d{h}", [18, WL], BF16, kind="Internal")
             for h in range(2)]

    with tile.TileContext(nc) as tc:
        with tc.tile_pool(name="singles", bufs=1) as singles:
            # small constants first so compute can start while xh streams in
            wb_sb = singles.tile([128, 26], FP32)
            nc.sync.dma_start(out=wb_sb, in_=wb.ap())
            pw_sb = singles.tile([128, 4], BF16)
            nc.sync.dma_start(out=pw_sb, in_=pw.ap())
            id_sb = singles.tile([128, 128], BF16)
            nc.sync.dma_start(out=id_sb, in_=ident.ap())
            nsw_sb = singles.tile([128, 128], BF16)
            nc.sync.dma_start(out=nsw_sb, in_=nsw.ap())
            dg_sb = singles.tile([128, len(PE_TAPS), 128], BF16)
            nc.sync.dma_start(out=dg_sb,
                              in_=dg.ap().rearrange("(k p) m -> p k m", p=128))
            xh_sb = singles.tile([128, 132, 260], BF16)
            for ci in range(11):
                r0c, r1c = ci * 12, min(132, ci * 12 + 12)
                eng = (nc.sync, nc.scalar)[ci % 2]
                eng.dma_start(out=xh_sb[:, r0c:r1c, :],
                              in_=xh.ap()[:, r0c:r1c, :])
            eps_t = singles.tile([128, 1], FP32)
            nc.vector.memset(eps_t, 1e-5)
            sp = singles.tile([128, NT], FP32)
            s2p = singles.tile([128, NT], FP32)
            stat = singles.tile([128, 8], FP32)

            # ---------------- Phase A: conv + stats; Y stored bf16 -------------
            with tc.tile_pool(name="ab", bufs=1) as ab:
              y_bf = ab.tile([128, FH // 2], BF16)
              with tc.tile_pool(name="pha", bufs=2) as pha, \
                   tc.tile_pool(name="psA", bufs=2, space="PSUM") as psA:
                for t in range(NT):
                    r0 = t * RT

                    def sl(dy, dx, r0=r0):
                        return xh_sb[:, r0 + dy:r0 + RT + dy, dx:dx + W]

                    # DVE tap pairs: ts product x2 (4x mode) + TT pair-add,
                    # each pair merged into psum by one PE identity pass
                    ps = psA.tile([128, FT], FP32)
                    nch = FT // 512
                    # ACT writes the first ACT tap product straight into psum
                    # (initializes the accumulation region; all matmuls then
                    # accumulate with start=False)
                    ka0 = ACT_TAPS[0]
                    nc.scalar.mul(out=ps, in_=sl(*divmod(ka0, 5)),
                                  mul=wb_sb[:, ka0:ka0 + 1])
                    for ki, k in enumerate(PE_TAPS):
                        dy, dx = divmod(k, 5)
                        for j in range(nch):
                            nc.tensor.matmul(
                                ps[:, j * 512:(j + 1) * 512], dg_sb[:, ki, :],
                                xh_sb[:, r0 + dy + 2 * j:r0 + dy + 2 * j + 2,
                                      dx:dx + W],
                                start=False, stop=False, skip_group_check=True)

                    def pe_merge(src3d, last=False):
                        for j in range(nch):
                            nc.tensor.matmul(
                                ps[:, j * 512:(j + 1) * 512], id_sb,
                                src3d[:, 2 * j:2 * j + 2, :],
                                start=False, stop=(last and j == nch - 1))

                    npair = len(DVE_TAPS) // 2
                    pairs = []
                    for pi in range(npair):
                        ka, kb = DVE_TAPS[2 * pi], DVE_TAPS[2 * pi + 1]
                        pa = pha.tile([128, RT, W], BF16, tag=f"pa{pi % 2}",
                                      bufs=2)
                        pb = pha.tile([128, RT, W], BF16, tag="pb", bufs=2)
                        # fold the conv bias into the first product
                        if pi == 0:
                            nc.vector.tensor_scalar(
                                out=pa, in0=sl(*divmod(ka, 5)),
                                scalar1=wb_sb[:, ka:ka + 1],
                                scalar2=wb_sb[:, 25:26],
                                op0=AL.mult, op1=AL.add)
                        else:
                            nc.vector.tensor_scalar(
                                out=pa, in0=sl(*divmod(ka, 5)),
                                scalar1=wb_sb[:, ka:ka + 1], scalar2=None,
                                op0=AL.mult)
                        nc.vector.tensor_scalar(
                            out=pb, in0=sl(*divmod(kb, 5)),
                            scalar1=wb_sb[:, kb:kb + 1], scalar2=None, op0=AL.mult)
                        if pi < npair - 4:
                            nc.vector.tensor_tensor(out=pa, in0=pa, in1=pb,
                                                    op=AL.add)
                            pairs.append(pa)
                        else:
                            pairs.append(pa)
                            pairs.append(pb)
                    for m in pairs:
                        pe_merge(m)

                    # remaining ACT products: one pair (GPS-added), one single
                    # folded into the GPS-tap chain below
                    aps = []
                    for ai, k in enumerate(ACT_TAPS[1:]):
                        at = pha.tile([128, RT, W], BF16, tag=f"at{ai % 2}")
                        nc.scalar.mul(out=at, in_=sl(*divmod(k, 5)),
                                      mul=wb_sb[:, k:k + 1])
                        aps.append(at)
                    nc.gpsimd.tensor_tensor(out=aps[0], in0=aps[0], in1=aps[1],
                                            op=AL.add)
                    pe_merge(aps[0])

                    # GPSIMD ts pair, DVE-added; ACT single joins via GPS add
                    ka, kb = GPS_TAPS
                    ga = pha.tile([128, RT, W], BF16, tag="ga")
                    gb = pha.tile([128, RT, W], BF16, tag="gb")
                    nc.gpsimd.tensor_scalar(out=ga, in0=sl(*divmod(ka, 5)),
                                            scalar1=wb_sb[:, ka:ka + 1],
                                            scalar2=None, op0=AL.mult)
                    nc.gpsimd.tensor_scalar(out=gb, in0=sl(*divmod(kb, 5)),
                                            scalar1=wb_sb[:, kb:kb + 1],
                                            scalar2=None, op0=AL.mult)
                    nc.vector.tensor_tensor(out=ga, in0=ga, in1=gb, op=AL.add)
                    nc.gpsimd.tensor_tensor(out=ga, in0=ga, in1=aps[2],
                                            op=AL.add)
                    pe_merge(ga, last=True)

                    # stats from psum + store bf16 (Copy pass writes Y;
                    # Square scratch writes the same region first, WAW-ordered).
                    # Second half spills to DRAM to free SBUF for phase C.
                    if t < NT // 2:
                        ydst = y_bf[:, t * FT:(t + 1) * FT]
                    else:
                        ydst = pha.tile([128, FT], BF16, tag="ysp", bufs=2)
                    nc.scalar.activation(out=ydst, in_=ps, func=AF.Square,
                                         accum_out=s2p[:, t:t + 1])
                    nc.scalar.activation(out=ydst, in_=ps, func=AF.Copy,
                                         accum_out=sp[:, t:t + 1])
                    if t >= NT // 2:
                        tl = t - NT // 2
                        nc.sync.dma_start(
                            out=y_d.ap()[:, tl * FT:(tl + 1) * FT], in_=ydst)

              # ------------- stats combine (collective over the pair) -----------
              nc.vector.tensor_reduce(out=stat[:, 0:1], in_=sp, axis=AX.X, op=AL.add)
              nc.vector.tensor_reduce(out=stat[:, 1:2], in_=s2p, axis=AX.X, op=AL.add)
              nc.sync.dma_start(out=cc_in.ap(), in_=stat[:, 0:2])
              if not timing:
                  nc.gpsimd.collective_compute(
                      kind="AllReduce", op=AL.add,
                      replica_groups=[[0, 1], [2, 3], [4, 5], [6, 7]],
                      ins=[cc_in.ap()], outs=[cc_out.ap()])
              else:
                  nc.sync.dma_start(out=cc_out.ap(), in_=cc_in.ap())
              nc.sync.dma_start(out=stat[:, 0:2], in_=cc_out.ap())
              inv = 1.0 / (H * W)
              nc.vector.tensor_scalar(out=stat[:, 2:3], in0=stat[:, 0:1],
                                      scalar1=inv, scalar2=None, op0=AL.mult)
              nc.vector.tensor_scalar(out=stat[:, 3:4], in0=stat[:, 1:2],
                                      scalar1=inv, scalar2=None, op0=AL.mult)
              nc.vector.scalar_tensor_tensor(
                  out=stat[:, 4:5], in0=stat[:, 2:3], scalar=stat[:, 2:3],
                  in1=stat[:, 3:4], op0=AL.mult, op1=AL.subtract)
              nc.scalar.activation(out=stat[:, 5:6], in_=stat[:, 4:5],
                                   func=AF.Sqrt, scale=-1.0, bias=eps_t[:, 0:1])
              nc.vector.reciprocal(out=stat[:, 6:7], in_=stat[:, 5:6])
              nc.vector.tensor_scalar(out=stat[:, 7:8], in0=stat[:, 2:3],
                                      scalar1=stat[:, 6:7], scalar2=-1.0,
                                      op0=AL.mult, op1=AL.mult)

              # ------ Phases B (gelu+1x1->flow) and C (warp), per-half ------
              # pipelined: B(h0), maps(h0), then B(h1) tile-interleaved with
              # C(h0), maps(h1), C(h1).
              taps = [(sy, sx) for sy in (-1, 0, 1) for sx in (-1, 0, 1)]
              with tc.tile_pool(name="phb", bufs=2) as phb, \
                   tc.tile_pool(name="cw", bufs=2) as cw, \
                   tc.tile_pool(name="phc", bufs=2) as phc:
                p9_sb = {}
                sel_sb = cw.tile([64, 32, 128], BF16, tag="sel", bufs=1)
                nc.sync.dma_start(out=sel_sb, in_=sel.ap())

                def b_tile(t, psf_pool, cwid):
                    h, tl = t // (NT // 2), t % (NT // 2)
                    if h == 0:
                        ysrc = y_bf[:, t * FT:(t + 1) * FT]
                    else:
                        ysrc = phb.tile([128, FT], BF16, tag="yin", bufs=2)
                        nc.sync.dma_start(
                            out=ysrc, in_=y_d.ap()[:, tl * FT:(tl + 1) * FT])
                    g = phb.tile([128, FT], BF16, tag="g")
                    nc.scalar.activation(out=g, in_=ysrc,
                                         func=AF.Gelu, scale=stat[:, 6:7],
                                         bias=stat[:, 7:8])
                    fls = phb.tile([4, FT], BF16, tag="fls", bufs=1)
                    for j in range(FT // cwid):
                        psf = psf_pool.tile([4, cwid], FP32, tag="psf")
                        for jj in range(cwid // 512):
                            nc.tensor.matmul(
                                psf[:, jj * 512:(jj + 1) * 512], pw_sb,
                                g[:, j * cwid + jj * 512:
                                  j * cwid + (jj + 1) * 512],
                                start=True, stop=True)
                        if h == 0 and j % 2 == 0:
                            nc.vector.tensor_copy(
                                out=fls[:, j * cwid:(j + 1) * cwid], in_=psf)
                        else:
                            nc.scalar.copy(
                                out=fls[:, j * cwid:(j + 1) * cwid], in_=psf)
                    nc.scalar.dma_start(
                        out=flow_hs[h].ap()[:, tl * FT:(tl + 1) * FT], in_=fls)

                def maps_half(h):
                    # compact maps [64, FC]: partitions 0-31 field1, 32-63
                    # field2; pixel = partition*512 + free (row-major 256)
                    FC = FH2 // 32
                    cx = cw.tile([64, FC], BF16, tag="cx", bufs=1)
                    cy = cw.tile([64, FC], BF16, tag="cy", bufs=1)
                    for (dst, r1, r2) in ((cx, 0, 2), (cy, 1, 3)):
                        for (p0, row) in ((0, r1), (32, r2)):
                            nc.scalar.dma_start(
                                out=dst[p0:p0 + 32, :],
                                in_=flow_hs[h].ap()[row:row + 1, :].rearrange(
                                    "a (p f) -> (a p) f", p=32))
                    wsel = {}
                    for ax, d in (("x", cx), ("y", cy)):
                        wp = cw.tile([64, FC], BF16, tag=f"wp{ax}", bufs=1)
                        wm = cw.tile([64, FC], BF16, tag=f"wm{ax}", bufs=1)
                        w0 = cw.tile([64, FC], BF16, tag=f"w0{ax}", bufs=1)
                        nc.vector.tensor_scalar(out=wp, in0=d, scalar1=0.0,
                                                scalar2=None, op0=AL.max)
                        nc.vector.tensor_scalar(out=wm, in0=d, scalar1=-1.0,
                                                scalar2=0.0, op0=AL.mult,
                                                op1=AL.max)
                        nc.scalar.activation(out=w0, in_=d, func=AF.Abs)
                        nc.vector.tensor_scalar(out=w0, in0=w0, scalar1=-1.0,
                                                scalar2=1.0, op0=AL.mult,
                                                op1=AL.add)
                        wsel[ax] = {-1: wm, 0: w0, 1: wp}
                    wd = w9_hs[h].ap()
                    for ki, (sy, sx) in enumerate(taps):
                        tag = f"p9k{ki}" if ki in PE_BCAST else "p9"
                        p9 = cw.tile([64, FC], BF16, tag=tag)
                        nc.vector.tensor_tensor(out=p9, in0=wsel["y"][sy],
                                                in1=wsel["x"][sx], op=AL.mult)
                        if ki in PE_BCAST:
                            # stays in SBUF; broadcast via PE selector matmul
                            p9_sb[(h, ki)] = p9
                        else:
                            dst = bass.AP(tensor=wd.tensor, offset=ki * WL,
                                          ap=[[9 * WL, 2], [512, 32], [1, 512]])
                            nc.scalar.dma_start(out=dst, in_=p9)

                wt_bc = {}

                def bc_prep(t):
                    # PE selector-matmul broadcast of the SBUF-resident maps
                    # for tile t (emitted one tile early so the PE work sits
                    # ahead of the previous tile's merges in the queue)
                    h, tl = t // (NT // 2), t % (NT // 2)
                    nch = FT // 512
                    for ki in sorted(PE_BCAST):
                        wt = phc.tile([128, RT, W], BF16, tag=f"wb{ki}",
                                      bufs=2)
                        p9k = p9_sb[(h, ki)]
                        for j in range(nch):
                            q = tl * nch + j
                            psm = psS.tile([128, 512], FP32, tag="psm")
                            nc.tensor.matmul(psm, sel_sb[:, q, :], p9k,
                                             start=True, stop=True)
                            nc.scalar.copy(
                                out=wt[:, 2 * j:2 * j + 2, :], in_=psm)
                        wt_bc[(t, ki)] = wt

                def c_tile(t):
                    h, tl = t // (NT // 2), t % (NT // 2)
                    w9 = w9_hs[h].ap()
                    r0 = t * RT
                    acc = psC.tile([128, FT], FP32)
                    nch = FT // 512

                    # 9 taps: map tile arrives either by DMA broadcast-read
                    # from DRAM or by the prefetched PE selector-broadcast;
                    # multiply the shifted image window in place, merge via
                    # PE identity
                    dpos = 0
                    for ki in range(9):
                        sy, sx = taps[ki]
                        if ki in PE_BCAST:
                            wt = wt_bc.pop((t, ki))
                        else:
                            wt = phc.tile([128, RT, W], BF16,
                                          tag=f"w9t{dpos % 3}", bufs=2)
                            dpos += 1
                            src = bass.AP(tensor=w9.tensor,
                                          offset=ki * WL + tl * FT,
                                          ap=[[9 * WL, 2], [0, 64], [1, FT]])
                            eng = {0: nc.sync, 3: nc.sync, 1: nc.sync,
                                   4: nc.sync}.get(ki, nc.gpsimd)
                            eng.dma_start(out=wt, in_=src)
                        nc.vector.tensor_tensor(
                            out=wt, in0=wt,
                            in1=xh_sb[:, r0 + 2 + sy:r0 + 2 + RT + sy,
                                      2 + sx:2 + sx + W],
                            op=AL.mult)
                        for j in range(nch):
                            nc.tensor.matmul(
                                acc[:, j * 512:(j + 1) * 512], id_sb,
                                wt[:, 2 * j:2 * j + 2, :],
                                start=(ki == 0), stop=False)
                    # subtract swapped-half center via permuted negative identity
                    for j in range(nch):
                        nc.tensor.matmul(acc[:, j * 512:(j + 1) * 512], nsw_sb,
                                         xh_sb[:, r0 + 2 + 2 * j:r0 + 4 + 2 * j,
                                               2:2 + W],
                                         start=False, stop=True)
                    outs = phc.tile([128, FT], BF16, tag="outs", bufs=1)
                    nc.scalar.copy(out=outs, in_=acc)
                    nc.gpsimd.dma_start(out=out_d.ap()[:, t * FT:(t + 1) * FT],
                                        in_=outs)

                NH = NT // 2
                with tc.tile_pool(name="psS", bufs=2, space="PSUM") as psS, \
                     tc.tile_pool(name="psC", bufs=1, space="PSUM") as psC:
                    for tl in range(NH):
                        b_tile(tl, psS, 512)
                    maps_half(0)
                    bc_prep(0)
                    for tl in range(NH - 1):
                        b_tile(NH + tl, psS, 512)
                        bc_prep(tl + 1)
                        c_tile(tl)
                    b_tile(2 * NH - 1, psS, 512)
                    maps_half(1)
                    bc_prep(NH)
                    c_tile(NH - 1)
                    for tl in range(NH, NT - 1):
                        bc_prep(tl + 1)
                        c_tile(tl)
                    c_tile(NT - 1)
    nc.compile()
    return nc


def _prep_inputs(x1, x2, dw_w, dw_b, pw_w):
    bf = ml_dtypes.bfloat16
    xcat = np.concatenate([x1, x2], axis=1)  # [N,128,H,W] f32
    xpad = np.pad(xcat, ((0, 0), (0, 0), (2, 2), (2, 2))).astype(bf)
    wb = np.concatenate([dw_w.reshape(128, 25), dw_b.reshape(128, 1)],
                        axis=1).astype(np.float32)
    pwm = (pw_w.reshape(4, 128).T * DSCALE).astype(bf)  # [128,4]
    dgm = np.zeros((len(PE_TAPS) * 128, 128), dtype=bf)
    for ki, k in enumerate(PE_TAPS):
        dy, dx = divmod(k, 5)
        np.fill_diagonal(dgm[ki * 128:(ki + 1) * 128], dw_w[:, 0, dy, dx].astype(bf))
    idm = np.eye(128, dtype=bf)
    nswm = np.zeros((128, 128), dtype=bf)
    for m in range(128):
        nswm[(m + 64) % 128, m] = -1.0
    selm = np.zeros((64, 32, 128), dtype=bf)
    for q in range(32):
        selm[q, q, 0:64] = 1.0
        selm[q + 32, q, 64:128] = 1.0
    selm = selm.reshape(64, 32 * 128)
    in_maps = []
    for c in range(8):
        n, s = c // 2, c % 2
        in_maps.append({
            "xh": np.ascontiguousarray(xpad[n, :, 128 * s:128 * s + 132, :]),
            "wb": wb, "pw": pwm, "dg": dgm, "ident": idm, "nsw": nswm,
            "sel": selm,
        })
    return in_maps


def _run(x1, x2, dw_w, dw_b, pw_w, trace=False):
    if "nc" not in _CACHE:
        _CACHE["nc"] = _build()
    in_maps = _prep_inputs(np.asarray(x1, np.float32), np.asarray(x2, np.float32),
                           np.asarray(dw_w, np.float32), np.asarray(dw_b, np.float32),
                           np.asarray(pw_w, np.float32))
    res = run_bass_kernel_spmd(_CACHE["nc"], in_maps, core_ids=list(range(8)),
                               trace=trace)
    o1 = np.empty((N, C, H, W), np.float32)
    o2 = np.empty((N, C, H, W), np.float32)
    for c in range(8):
        n, s = c // 2, c % 2
        o = res.results[c]["out"].astype(np.float32).reshape(128, HH, W)
        o1[n, :, 128 * s:128 * (s + 1), :] = o[:64]
        o2[n, :, 128 * s:128 * (s + 1), :] = o[64:]
    return (o1, o2), res


def kernel(x1, x2, dw_w, dw_b, pw_w):
    (o1, o2), _ = _run(x1, x2, dw_w, dw_b, pw_w, trace=False)
    return (o1, o2)

